# revision 1
# baseline (speedup 1.0000x reference)
"""Trainium2 Bass kernel for nn_DELTAModel (3-layer edge-conditioned graph
attention, N=50k / E=1.6M), SPMD across 8 NeuronCores.

Sharding: edge-cut by dst node. Core c owns nodes [c*6250,(c+1)*6250) and all
edges whose dst lands there, sorted by dst, padded per (128-node block,
src-half section) to a chunk structure identical across cores (one shared
instruction stream; per-core differences live in DRAM contents only).

Per-edge data is fetched with gpsimd.dma_gather (int16 indices -> node tables
split at row 32768 into low/high sections; 256-byte rows).

Softmax: shift-invariant, no segment max (scores are O(1)); normalization
deferred to node level: agg = segsum(p*v) / (segsum(p)+1e-9).
Segment sums: one-hot matmuls per 128-edge chunk accumulated in PSUM per
dst block.  h stays SBUF-resident; after each node update, cores all-gather
bf16 h and redundantly rebuild gather tables.  Edge state e is stored as
un-affined LN output; the (ge,be) affine is folded into consumer weights.
"""

import os
import sys

for _p in ("/root/pylib", "/opt/trn_rl_repo"):
    if os.path.isdir(_p) and _p not in sys.path:
        sys.path.append(_p)

import numpy as np
import ml_dtypes

import concourse.bacc as bacc
import concourse.mybir as mybir
import concourse.tile as tile
from concourse.bass import ts
from concourse.bass_utils import run_bass_kernel_spmd
from concourse.masks import make_identity

BF16 = mybir.dt.bfloat16
F32 = mybir.dt.float32
I16 = mybir.dt.int16
AF = mybir.ActivationFunctionType
ALU = mybir.AluOpType
AX = mybir.AxisListType


class Cfg:
    def __init__(self, N=50000, E=1600000, D=64, DE=32, H=4, L=3, C=8,
                 SPLIT=32768, GBLK=2, WGC=4):
        self.N, self.E, self.D, self.DE, self.H, self.L, self.C = N, E, D, DE, H, L, C
        self.SPLIT = SPLIT
        self.GBLK = GBLK
        self.WGC = WGC
        self.NLOC = N // C
        assert self.NLOC * C == N
        self.NBLK = (self.NLOC + 127) // 128
        self.NLOCP = self.NBLK * 128


# ------------------------------------------------------------ preprocessing --
def _wrap_idx(a):
    n = a.shape[0]
    w = a.reshape(n // 16, 16).T.astype(np.int16)
    return np.tile(w, (8, 1))


def _bf(x):
    return np.asarray(x).astype(ml_dtypes.bfloat16)


class Plan:
    def __init__(self, cfg, edge_index, edge_features):
        C, NLOC, SPLIT = cfg.C, cfg.NLOC, cfg.SPLIT
        src = np.asarray(edge_index[0]).astype(np.int64)
        dst = np.asarray(edge_index[1]).astype(np.int64)
        core = dst // NLOC
        self.cfg = cfg
        nchunk = np.zeros((cfg.NBLK, 2), np.int64)
        per_core = []
        for c in range(C):
            ids = np.nonzero(core == c)[0]
            dl = dst[ids] - c * NLOC
            sec = (src[ids] >= SPLIT).astype(np.int64)
            order = np.lexsort((dl, sec, dl // 128))
            ids, dl, sec = ids[order], dl[order], sec[order]
            blk = dl // 128
            per_core.append((ids, dl, sec, blk))
            for b in range(cfg.NBLK):
                m = blk == b
                for s in (0, 1):
                    cnt = int(np.count_nonzero(m & (sec == s)))
                    nchunk[b, s] = max(nchunk[b, s], (cnt + 127) // 128)
        nchunk = ((nchunk + 3) // 4) * 4  # WG=512: runs %512
        self.nchunk = nchunk
        self.slots = nchunk * 128
        self.EP = int(self.slots.sum())
        assert self.EP > 0

        off = np.zeros((cfg.NBLK, 2), np.int64)
        t = 0
        for b in range(cfg.NBLK):
            for s in (0, 1):
                off[b, s] = t
                t += self.slots[b, s]
        self.off = off

        self.groups = [(b0, min(b0 + cfg.GBLK, cfg.NBLK))
                       for b0 in range(0, cfg.NBLK, cfg.GBLK)]

        ef = np.asarray(edge_features)
        self.srcidx, self.dstloc, self.dstcol, self.e0, self.edge_pos = \
            [], [], [], [], []
        for c in range(C):
            ids, dl, sec, blk = per_core[c]
            si = np.zeros(self.EP, np.int16)
            di = np.zeros(self.EP, np.int16)
            dc = np.full(self.EP, -1.0, np.float32)
            ep = np.full(self.EP, -1, np.int64)
            for b in range(cfg.NBLK):
                for s in (0, 1):
                    m = (blk == b) & (sec == s)
                    k = int(np.count_nonzero(m))
                    o = off[b, s]
                    si[o:o + k] = (src[ids[m]] - (SPLIT if s else 0)).astype(np.int16)
                    di[o:o + k] = dl[m].astype(np.int16)
                    dc[o:o + k] = (dl[m] - 128 * b).astype(np.float32)
                    ep[o:o + k] = ids[m]
            e0 = np.zeros((self.EP, cfg.DE), np.float32)
            real = ep >= 0
            e0[real] = ef[ep[real]]
            e0 = (e0.reshape(self.EP // 512, 4, 128, cfg.DE)
                  .transpose(0, 2, 1, 3).reshape(self.EP // 4, 128))
            self.srcidx.append(si)
            self.dstloc.append(di)
            self.dstcol.append(dc)
            self.e0.append(_bf(e0))
            self.edge_pos.append(ep)

        # per-(group) metadata
        self.gmeta = []
        for (b0, b1) in self.groups:
            n_s = [int(self.slots[b0:b1, s].sum()) for s in (0, 1)]
            chunks = []
            colpos = [0, 0]
            for b in range(b0, b1):
                for s in (0, 1):
                    for _ in range(int(self.nchunk[b, s])):
                        chunks.append((b, s, colpos[s]))
                        colpos[s] += 1
            self.gmeta.append(dict(b0=b0, b1=b1, n=n_s, chunks=chunks,
                                   stream_off=int(off[b0, 0]),
                                   stream_len=int(self.slots[b0:b1].sum())))
        # chunks per block (for PSUM stop flags)
        self.blk_chunks = [int(nchunk[b, 0] + nchunk[b, 1])
                           for b in range(cfg.NBLK)]

        def build_wrapped(idx_arrays):
            outs = []
            for c in range(C):
                parts = [[], []]
                for gm in self.gmeta:
                    for s in (0, 1):
                        if not gm["n"][s]:
                            continue
                        seg = []
                        for b in range(gm["b0"], gm["b1"]):
                            o = off[b, s]
                            seg.append(idx_arrays[c][o:o + self.slots[b, s]])
                        parts[s].append(_wrap_idx(np.concatenate(seg)))
                outs.append([np.concatenate(p, axis=1) if p else
                             np.zeros((128, 1), np.int16) for p in parts])
            return outs

        self.src_widx = build_wrapped(self.srcidx)
        self.dst_widx = []
        for c in range(C):
            parts = [_wrap_idx(self.dstloc[c][gm["stream_off"]:
                                              gm["stream_off"] + gm["stream_len"]])
                     for gm in self.gmeta if gm["stream_len"]]
            self.dst_widx.append(np.concatenate(parts, axis=1) if parts else
                                 np.zeros((128, 1), np.int16))


def fold_weights(cfg, W):
    L, D, DE = cfg.L, cfg.D, cfg.DE
    out = {}
    for l in range(L):
        g_prev = np.ones(DE, np.float32) if l == 0 else np.asarray(W["ge"][l - 1], np.float32)
        b_prev = np.zeros(DE, np.float32) if l == 0 else np.asarray(W["be"][l - 1], np.float32)
        We = np.asarray(W["We"][l], np.float32)
        WeFl = g_prev[:, None] * We
        bd = np.zeros((128, 16), np.float32)
        for a in range(4):
            bd[32 * a:32 * a + 32, 4 * a:4 * a + 4] = WeFl
        out[f"WeF_{l}"] = _bf(bd)                                        # [128,16]
        Wem = np.asarray(W["Wem"][l], np.float32)
        A, B, Cm = Wem[:D], Wem[D:2 * D], Wem[2 * D:]
        bem_f = np.asarray(W["bem"][l], np.float32) + b_prev @ Cm
        out[f"WemA_{l}"] = _bf(np.concatenate([A, bem_f[None, :]], 0))   # [65,32]
        Cf = g_prev[:, None] * Cm
        bd = np.zeros((128, 128), np.float32)
        for a in range(4):
            bd[32 * a:32 * a + 32, 32 * a:32 * a + 32] = Cf
        out[f"WemC_{l}"] = _bf(bd)                                       # [128,128]
        bd = np.zeros((128, 128), np.float32)
        for a in range(4):
            bd[32 * a:32 * a + 32, 32 * a:32 * a + 32] = np.diag(g_prev)
        out[f"WemI_{l}"] = _bf(bd)                                       # [128,128]
        out[f"Be_{l}"] = b_prev[None, :].astype(np.float32)              # [1,32]
        out[f"WKV_{l}"] = _bf(np.concatenate(
            [np.asarray(W["Wk"][l]), np.asarray(W["Wv"][l])], 1))        # [64,128]
        zb = (np.zeros((D, DE), np.float32) if l == 0
              else np.asarray(W["Wem"][l - 1], np.float32)[D:2 * D])
        out[f"WQZB_{l}"] = _bf(np.concatenate(
            [np.asarray(W["Wq"][l], np.float32) * 0.25, zb,
             np.zeros((D, D - DE), np.float32)], 1))                     # [64,128]
        out[f"Wo_{l}"] = _bf(np.asarray(W["Wo"][l]))
        out[f"gn_{l}"] = np.asarray(W["gn"][l], np.float32)[None, :]
        out[f"bn_{l}"] = np.asarray(W["bn"][l], np.float32)[None, :]
    return out


# ---------------------------------------------------------------- builder ----
class Builder:
    def __init__(self, cfg, plan):
        self.cfg, self.plan = cfg, plan
        self.nc = bacc.Bacc("TRN2")

    def declare(self):
        nc, cfg, plan = self.nc, self.cfg, self.plan
        dram = lambda n, s, d, k="ExternalInput": nc.dram_tensor(n, s, d, kind=k)
        self.in_h = dram("h_loc", [cfg.NLOCP, 64], F32)
        self.in_kv0 = dram("kv0", [cfg.N, 128], BF16)
        self.in_dstq0 = dram("dstq0", [cfg.NLOCP, 128], BF16)
        self.in_e0 = dram("e0", [plan.EP // 4, 128], BF16)
        self.in_hbfi = dram("hbfi", [cfg.NLOC, 128], BF16)  # ones-col template
        self.in_swl = dram("swl", list(plan.src_widx[0][0].shape), I16)
        self.in_swh = dram("swh", list(plan.src_widx[0][1].shape), I16)
        self.in_dw = dram("dw", list(plan.dst_widx[0].shape), I16)
        self.in_dcol = dram("dcol", [128, plan.EP // 128], BF16)
        self.in_iota = dram("iota", [128, 128], BF16)
        self.in_ones = dram("onesrow", [1, 128], F32)
        self.w = {}
        for l in range(cfg.L):
            for nm, sh, dt in [
                (f"WeF_{l}", [128, 16], BF16), (f"WemA_{l}", [65, cfg.DE], BF16),
                (f"WemC_{l}", [128, 128], BF16),
                (f"WemI_{l}", [128, 128], BF16),
                (f"Be_{l}", [1, cfg.DE], F32),
                (f"WKV_{l}", [64, 128], BF16), (f"WQZB_{l}", [64, 128], BF16),
                (f"Wo_{l}", [64, 64], BF16),
                (f"gn_{l}", [1, 64], F32), (f"bn_{l}", [1, 64], F32),
            ]:
                self.w[nm] = dram(nm, sh, dt)
        self.out_h = dram("out", [cfg.NLOC, 64], F32, k="ExternalOutput")
        self.dbg = {}
        if os.environ.get("K_DEBUG_DUMP"):
            for nm, sh in [("dbg_hf0", [cfg.N, 128]), ("dbg_hf1", [cfg.N, 128]),
                           ("dbg_kv", [cfg.N, 128]),
                           ("dbg_dstq", [cfg.NLOCP, 128]),
                           ("dbg_hbf", [cfg.NLOC, 128])]:
                self.dbg[nm] = dram(nm, sh, BF16, k="ExternalOutput")

    def build(self):
        nc, cfg = self.nc, self.cfg
        self.declare()
        with tile.TileContext(nc, num_cores=cfg.C) as tc:
            self.tc = tc
            with tc.tile_pool(name="persist", bufs=1) as pp, \
                 tc.tile_pool(name="dramp", bufs=1, space="DRAM") as dp, \
                 tc.tile_pool(name="work", bufs=3) as wp, \
                 tc.tile_pool(name="gath", bufs=1) as gp:
                self.pp, self.dp, self.wp, self.gp = pp, dp, wp, gp
                self._persistent()
                self._internal_dram()
                kv, dstq, e_rd = self.in_kv0, self.in_dstq0, self.in_e0
                stop = os.environ.get("K_STOP", "")
                seq = []
                for l in range(cfg.L):
                    seq += [f"a{l}", f"n{l}"]
                    if l < cfg.L - 1:
                        seq += [f"t{l}", f"e{l}"]
                cut = seq.index(stop) if stop in seq else len(seq) - 1
                def _go(tag):
                    return seq.index(tag) <= cut
                barrier = (lambda: tc.strict_bb_all_engine_barrier()) \
                    if os.environ.get("K_BARRIER", "1") != "0" else (lambda: None)
                for l in range(cfg.L):
                    if _go(f"a{l}"):
                        self._a_pass(l, kv, dstq, e_rd)
                    if _go(f"n{l}"):
                        self._node_update(l)
                    if l < cfg.L - 1:
                        if _go(f"t{l}"):
                            self._tables(l + 1)
                            barrier()
                        e_wr = self.e_dev[l]
                        if _go(f"e{l}"):
                            self._e_pass(l, self.h_full, self.dstq_dev,
                                         e_rd, e_wr)
                            barrier()
                        kv, dstq, e_rd = self.kv_dev, self.dstq_dev, e_wr
                self._output()
                if self.dbg:
                    for nm, src in [("dbg_hf0", self.h_full_g[0]),
                                    ("dbg_hf1", self.h_full_g[1]),
                                    ("dbg_kv", self.kv_dev),
                                    ("dbg_dstq", self.dstq_dev),
                                    ("dbg_hbf", self.hbf_loc)]:
                        nc.sync.dma_start(out=self.dbg[nm][:], in_=src[:])
        nc.compile()
        return nc

    # ---- persistent state --------------------------------------------------
    def _persistent(self):
        nc, cfg = self.nc, self.cfg
        pp = self.pp
        qp_ctx = self.tc.tile_pool(name="ps_init", bufs=1, space="PSUM")
        qp = qp_ctx.__enter__()
        NBLK = cfg.NBLK
        self.h_sb = pp.tile([128, NBLK, 64], F32, name="h_sb")
        nc.sync.dma_start(
            out=self.h_sb[:],
            in_=self.in_h[:].rearrange("(t p) d -> p t d", p=128))
        self.U_sb = pp.tile([128, NBLK, 68], F32, name="U_sb")
        self.iota_sb = pp.tile([128, 128], BF16, name="iota_sb")
        nc.sync.dma_start(out=self.iota_sb[:], in_=self.in_iota[:])
        self.ident = pp.tile([128, 128], F32, name="ident")
        make_identity(nc, self.ident[:])
        self.ones_sb = pp.tile([1, 128], F32, name="ones_sb")
        nc.sync.dma_start(out=self.ones_sb[:], in_=self.in_ones[:])
        self.wsb = {}
        for nm, t in self.w.items():
            tt = pp.tile(list(t.shape), t.dtype, name=f"sb_{nm}")
            nc.sync.dma_start(out=tt[:], in_=t[:])
            self.wsb[nm] = tt
        self.GB = {}
        for l in range(cfg.L):
            for nm in (f"gn_{l}", f"bn_{l}", f"Be_{l}"):
                wdt = self.wsb[nm].shape[1]
                ps = qp.tile([128, wdt], F32, space="PSUM", name=f"ps_{nm}", tag="gbps")
                nc.tensor.matmul(ps[:], lhsT=self.ones_sb[:],
                                 rhs=self.wsb[nm][:], start=True, stop=True)
                sb = pp.tile([128, wdt], F32, name=f"gb_{nm}")
                nc.scalar.activation(sb[:], ps[:], AF.Copy)
                self.GB[nm] = sb
        self.dcol_sb = pp.tile([128, self.plan.EP // 128], BF16, name="dcol_sb")
        nc.sync.dma_start(out=self.dcol_sb[:], in_=self.in_dcol[:])
        qp_ctx.__exit__(None, None, None)

    def _internal_dram(self):
        cfg, dp, nc = self.cfg, self.dp, self.nc
        self.kv_dev = dp.tile([cfg.N, 128], BF16, name="kv_dev")
        self.dstq_dev = dp.tile([cfg.NLOCP, 128], BF16, name="dstq_dev")
        self.e_dev = [dp.tile([self.plan.EP // 4, 128], BF16,
                              name=f"e_dev{i}")
                      for i in range(max(cfg.L - 1, 1))]
        self.hbf_loc = dp.tile([cfg.NLOC, 128], BF16, name="hbf_loc")
        self.h_full_g = [dp.tile([cfg.N, 128], BF16, name=f"h_full{g}",
                                 addr_space="Shared")
                         for g in range(1, cfg.L)]
        self.h_full = None
        # ones-column template (h cols overwritten each gen)
        nc.sync.dma_start(out=self.hbf_loc[:], in_=self.in_hbfi[:])

    # ---- gather helper -----------------------------------------------------
    def _gather(self, table_ap, widx_t, woff, n, transpose, tag):
        nc, gp = self.nc, self.gp
        bufs = 2 if tag == "dg" else 3
        it = gp.tile([128, n // 16], I16, tag="gidx_" + tag, bufs=4,
                     name="gidx")
        nc.sync.dma_start(out=it[:], in_=widx_t[:, woff:woff + n // 16])
        if transpose:
            buf = gp.tile([128, 1, n], BF16, tag=tag, bufs=bufs, name="tgb")
            nc.gpsimd.dma_gather(buf[:], table_ap, it[:], n, n, 128,
                                 transpose=True, single_packet=False)
        else:
            buf = gp.tile([128, n // 128, 128], BF16, tag=tag, bufs=bufs,
                          name="gb")
            nc.gpsimd.dma_gather(buf[:], table_ap, it[:], n, n, 128,
                                 single_packet=False)
        return buf

    # ---- attention pass ----------------------------------------------------
    def _a_pass(self, l, kv_tab, dstq_tab, e_rd):
        cfg, plan = self.cfg, self.plan
        qp_ctx = self.tc.tile_pool(name=f"ps_a{l}", bufs=2, space="PSUM")
        self.qp = qp_ctx.__enter__()
        WeF = self.wsb[f"WeF_{l}"]
        soff = [0, 0]
        doff = 0
        self._blk_seen = {}
        for gm in plan.gmeta:
            sbuf = [None, None]
            for s in (0, 1):
                n = gm["n"][s]
                if n:
                    tab = (kv_tab[0:cfg.SPLIT, :] if s == 0 else
                           kv_tab[cfg.SPLIT:cfg.N, :])
                    widx = self.in_swl if s == 0 else self.in_swh
                    sbuf[s] = self._gather(tab, widx, soff[s], n, False, "sg")
                    soff[s] += n // 16
            nd = gm["stream_len"]
            dbuf = self._gather(dstq_tab[0:cfg.NLOCP, :], self.in_dw, doff,
                                nd, False, "dg")
            doff += nd // 16
            self._a_group(l, gm, sbuf, dbuf, e_rd, WeF)
        qp_ctx.__exit__(None, None, None)

    def _a_group(self, l, gm, sbuf, dbuf, e_rd, WeF):
        nc, cfg, plan = self.nc, self.cfg, self.plan
        wp, qp = self.wp, self.qp
        lvl = int(os.environ.get("K_ALVL", "9"))
        chunks = gm["chunks"]
        so = gm["stream_off"]
        assert len(chunks) % 4 == 0
        for w in range(len(chunks) // 4):
            wch = chunks[w * 4:(w + 1) * 4]
            base = so + w * 512
            if lvl < 2:
                continue
            eT = wp.tile([128, 128], BF16, tag="eT", name="eT")
            nc.sync.dma_start(out=eT[:], in_=e_rd[base // 4:base // 4 + 128, :],
                              transpose=True)
            bias_ps = qp.tile([128, 16], F32, space="PSUM", tag="bias",
                              name="bias_ps")
            nc.tensor.matmul(bias_ps[:], lhsT=eT[:], rhs=WeF[:],
                             start=True, stop=True)
            if lvl < 3:
                continue
            s_sb = wp.tile([128, 4, 4], F32, tag="s_sb", name="s_sb")
            qk = wp.tile([128, 4, 64], BF16, tag="qk", name="qk")
            O = wp.tile([128, 4, 128], BF16, tag="O", name="O")
            M = wp.tile([128, 4, 68], BF16, tag="M", name="M")
            p_sb = wp.tile([128, 4, 4], BF16, tag="p_sb", name="p_sb")
            dc = self.dcol_sb[:, base // 128: base // 128 + 4]
            nc.vector.tensor_tensor(
                out=O[:],
                in0=dc[:, :, None].to_broadcast([128, 4, 128]),
                in1=self.iota_sb[:, None, :].to_broadcast([128, 4, 128]),
                op=ALU.is_equal)
            if lvl < 4:
                continue
            for j, (b, sct, col) in enumerate(wch):
                dcol_i = w * 4 + j
                nc.vector.tensor_tensor(
                    out=qk[:, j, :], in0=dbuf[:, dcol_i, 0:64],
                    in1=sbuf[sct][:, col, 0:64], op=ALU.mult)
            nc.vector.tensor_reduce(
                out=s_sb[:],
                in_=qk[:].rearrange("p c (h x) -> p c h x", h=4),
                axis=AX.X, op=ALU.add)
            nc.vector.tensor_tensor(
                out=s_sb[:], in0=s_sb[:],
                in1=bias_ps[:].rearrange("p (c f) -> p c f", f=4),
                op=ALU.add)
            nc.scalar.activation(p_sb[:], s_sb[:], AF.Exp)
            if lvl < 5:
                continue
            nc.scalar.activation(M[:, :, 64:68], p_sb[:], AF.Copy)
            for j, (b, sct, col) in enumerate(wch):
                nc.vector.tensor_tensor(
                    out=M[:, j, 0:64].rearrange("p (h x) -> p h x", h=4),
                    in0=p_sb[:, j, :, None].to_broadcast([128, 4, 16]),
                    in1=sbuf[sct][:, col, 64:128].rearrange(
                        "p (h x) -> p h x", h=4),
                    op=ALU.mult)
            if lvl < 6:
                continue
            for j, (b, sct, col) in enumerate(wch):
                seen = self._blk_seen.get(b, 0)
                if seen == 0:
                    self._ups = qp.tile([128, 68], F32, space="PSUM", tag="U",
                                        bufs=2, name="ups")
                last = seen + 1 == plan.blk_chunks[b]
                nc.tensor.matmul(self._ups[:], lhsT=O[:, j, :], rhs=M[:, j, :],
                                 start=(seen == 0), stop=last)
                self._blk_seen[b] = seen + 1
                if last:
                    nc.scalar.activation(self.U_sb[:, b, :], self._ups[:],
                                         AF.Copy)

    # ---- node update -------------------------------------------------------
    def _node_update(self, l):
        nc, cfg = self.nc, self.cfg
        qp_ctx = self.tc.tile_pool(name=f"ps_n{l}", bufs=2, space="PSUM")
        self.qp = qp_ctx.__enter__()
        wp, qp = self.wp, self.qp
        NBLK = cfg.NBLK
        U, h = self.U_sb, self.h_sb
        rec = wp.tile([128, NBLK, 4], F32, tag="rec", name="rec", bufs=1)
        nc.vector.tensor_scalar(rec[:], U[:, :, 64:68], 1e-9, None, ALU.add)
        nc.vector.reciprocal(rec[:], rec[:])
        agg = wp.tile([128, NBLK, 64], F32, tag="agg", name="agg", bufs=1)
        nc.vector.tensor_tensor(
            out=agg[:].rearrange("p t (h x) -> p t h x", h=4),
            in0=U[:, :, 0:64].rearrange("p t (h x) -> p t h x", h=4),
            in1=rec[:, :, :, None].to_broadcast([128, NBLK, 4, 16]),
            op=ALU.mult)
        x = wp.tile([128, NBLK, 64], F32, tag="x_nu", name="x_nu", bufs=1)
        Wo = self.wsb[f"Wo_{l}"]
        for t in range(NBLK):
            tp = qp.tile([128, 128], F32, space="PSUM", tag="tp", name="tp")
            nc.tensor.transpose(tp[0:64, :], agg[:, t, :], self.ident[:])
            aggT = wp.tile([64, 128], BF16, tag="aggT", name="aggT")
            nc.scalar.activation(aggT[:], tp[0:64, :], AF.Copy)
            rp = qp.tile([128, 64], F32, space="PSUM", tag="rp", name="rp")
            nc.tensor.matmul(rp[:], lhsT=aggT[:], rhs=Wo[:], start=True,
                             stop=True)
            nc.vector.tensor_tensor(out=x[:, t, :], in0=rp[:], in1=h[:, t, :],
                                    op=ALU.add)
        self._layernorm(x[:], h[:], 64, self.GB[f"gn_{l}"],
                        self.GB[f"bn_{l}"], tagsfx="_nu", bufs=1)
        if l < cfg.L - 1:
            self.h_full = self.h_full_g[l]
            T0, rem = cfg.NLOC // 128, cfg.NLOC % 128
            if T0:
                nc.gpsimd.dma_start(
                    out=self.hbf_loc[0:T0 * 128, 0:64]
                        .rearrange("(t p) d -> p t d", p=128),
                    in_=h[:, 0:T0, :])
            if rem:
                nc.gpsimd.dma_start(
                    out=self.hbf_loc[T0 * 128:cfg.NLOC, 0:64],
                    in_=h[0:rem, T0, :])
            nc.gpsimd.collective_compute(
                "AllGather", ALU.bypass, ins=[self.hbf_loc[:]],
                outs=[self.h_full[:]], replica_groups=[list(range(cfg.C))])
            if os.environ.get("K_BARRIER", "1") != "0":
                self.tc.strict_bb_all_engine_barrier()
        qp_ctx.__exit__(None, None, None)

    def _layernorm(self, x, out, F, Gt, Bt, tagsfx="", bufs=3):
        """out = LN(x)*G+B over last axis; x/out APs [128, T, F]."""
        nc, wp = self.nc, self.wp
        T = x.shape[1]
        tg = lambda t: t + tagsfx
        m1 = wp.tile([128, T], F32, tag=tg("m1"), name="m1", bufs=bufs)
        m2 = wp.tile([128, T], F32, tag=tg("m2"), name="m2", bufs=bufs)
        sq = wp.tile([128, T, F], F32, tag=tg("sq"), name="sq", bufs=bufs)
        nc.vector.tensor_reduce(out=m1[:, :, None], in_=x, axis=AX.X,
                                op=ALU.add)
        nc.scalar.activation(sq[:], x, AF.Square)
        nc.vector.tensor_reduce(out=m2[:, :, None], in_=sq[:], axis=AX.X,
                                op=ALU.add)
        mean = wp.tile([128, T], F32, tag=tg("mean"), name="mean", bufs=bufs)
        nc.vector.tensor_scalar(mean[:], m1[:], 1.0 / F, None, ALU.mult)
        v = wp.tile([128, T], F32, tag=tg("vvar"), name="vvar", bufs=bufs)
        nc.vector.tensor_tensor(out=v[:], in0=mean[:], in1=m1[:], op=ALU.mult)
        nc.vector.tensor_tensor(out=v[:], in0=m2[:], in1=v[:], op=ALU.subtract)
        nc.vector.tensor_scalar(v[:], v[:], 1.0 / F, 1e-5, ALU.mult, ALU.add)
        nc.vector.reciprocal(v[:], v[:])
        rstd = wp.tile([128, T], F32, tag=tg("rstd"), name="rstd", bufs=bufs)
        nc.scalar.activation(rstd[:], v[:], AF.Sqrt)
        xc = sq  # reuse (sq dead after m2)
        nc.vector.tensor_tensor(out=xc[:], in0=x,
                                in1=mean[:, :, None].to_broadcast([128, T, F]),
                                op=ALU.subtract)
        nc.vector.tensor_tensor(out=xc[:], in0=xc[:],
                                in1=rstd[:, :, None].to_broadcast([128, T, F]),
                                op=ALU.mult)
        if Gt is None:
            nc.vector.tensor_copy(out, xc[:])
        else:
            nc.vector.tensor_tensor(
                out=xc[:], in0=xc[:],
                in1=Gt[:, None, :].to_broadcast([128, T, F]), op=ALU.mult)
            nc.vector.tensor_tensor(
                out=out, in0=xc[:],
                in1=Bt[:, None, :].to_broadcast([128, T, F]), op=ALU.add)

    # ---- gather-table rebuild ---------------------------------------------
    def _tables(self, g):
        nc, cfg = self.nc, self.cfg
        qp_ctx = self.tc.tile_pool(name=f"ps_t{g}", bufs=2, space="PSUM")
        self.qp = qp_ctx.__enter__()
        wp, qp = self.wp, self.qp
        WKV = self.wsb[f"WKV_{g}"]
        WQZB = self.wsb[f"WQZB_{g}"]
        nfull, nrem = cfg.N // 128, cfg.N % 128
        for t in range(nfull + (1 if nrem else 0)):
            rows = nrem if t == nfull else 128
            hT = wp.tile([128, 128], BF16, tag="hT_tab", name="hT")
            nc.sync.dma_start(out=hT[:, 0:rows],
                              in_=self.h_full[t * 128:t * 128 + rows, :],
                              transpose=True)
            ps = qp.tile([128, 128], F32, space="PSUM", tag="kv_ps",
                         name="kv_ps")
            nc.tensor.matmul(ps[0:rows, :], lhsT=hT[0:64, 0:rows], rhs=WKV[:],
                             start=True, stop=True)
            kvt = wp.tile([128, 128], BF16, tag="kvt", name="kvt")
            nc.scalar.activation(kvt[0:rows, :], ps[0:rows, :], AF.Copy)
            nc.sync.dma_start(out=self.kv_dev[t * 128:t * 128 + rows, :],
                              in_=kvt[0:rows, :])
        for t in range(cfg.NBLK):
            tp = qp.tile([128, 128], F32, space="PSUM", tag="tp", name="tp")
            nc.tensor.transpose(tp[0:64, :], self.h_sb[:, t, :], self.ident[:])
            hTl = wp.tile([64, 128], BF16, tag="hT_tab", name="hTl")
            nc.scalar.activation(hTl[:], tp[0:64, :], AF.Copy)
            ps = qp.tile([128, 128], F32, space="PSUM", tag="kv_ps",
                         name="q_ps")
            nc.tensor.matmul(ps[:], lhsT=hTl[:], rhs=WQZB[:], start=True,
                             stop=True)
            qt = wp.tile([128, 128], BF16, tag="kvt", name="qt")
            nc.scalar.activation(qt[:], ps[:], AF.Copy)
            nc.sync.dma_start(out=self.dstq_dev[ts(t, 128), :], in_=qt[:])
        qp_ctx.__exit__(None, None, None)

    # ---- edge update pass --------------------------------------------------
    def _e_pass(self, l, htab, dstq_tab, e_rd, e_wr):
        cfg, plan = self.cfg, self.plan
        qp_ctx = self.tc.tile_pool(name=f"ps_e{l}", bufs=2, space="PSUM")
        self.qp = qp_ctx.__enter__()
        WemA = self.wsb[f"WemA_{l}"]
        WemC = self.wsb[f"WemC_{l}"]
        WemI = self.wsb[f"WemI_{l}"]
        soff = [0, 0]
        doff = 0
        for gm in plan.gmeta:
            sbuf = [None, None]
            for s in (0, 1):
                n = gm["n"][s]
                if n:
                    tab = (htab[0:cfg.SPLIT, :] if s == 0 else
                           htab[cfg.SPLIT:cfg.N, :])
                    widx = self.in_swl if s == 0 else self.in_swh
                    sbuf[s] = self._gather(tab, widx, soff[s], n, True, "sg")
                    soff[s] += n // 16
            nd = gm["stream_len"]
            dbuf = self._gather(dstq_tab[0:cfg.NLOCP, :], self.in_dw, doff,
                                nd, False, "dg")
            doff += nd // 16
            self._e_group(l, gm, sbuf, dbuf, e_rd, e_wr, WemA, WemC, WemI)
        qp_ctx.__exit__(None, None, None)

    def _e_group(self, l, gm, sbuf, dbuf, e_rd, e_wr, WemA, WemC, WemI):
        nc, cfg = self.nc, self.cfg
        wp, qp = self.wp, self.qp
        chunks = gm["chunks"]
        so = gm["stream_off"]
        scol = [0, 0]
        Be = self.GB[f"Be_{l}"]
        for w in range(len(chunks) // 4):
            wch = chunks[w * 4:(w + 1) * 4]
            base = so + w * 512
            eT = wp.tile([128, 128], BF16, tag="eT", name="eT")
            nc.sync.dma_start(out=eT[:], in_=e_rd[base // 4:base // 4 + 128, :],
                              transpose=True)
            z_ps = qp.tile([128, 128], F32, space="PSUM", tag="z_ps",
                           name="z_ps")
            em_ps = qp.tile([128, 128], F32, space="PSUM", tag="em_ps",
                            name="em_ps")
            nc.tensor.matmul(z_ps[:], lhsT=eT[:], rhs=WemC[:],
                             start=True, stop=False)
            nc.tensor.matmul(em_ps[:], lhsT=eT[:], rhs=WemI[:],
                             start=True, stop=True)
            for j, (b, sct, col) in enumerate(wch):
                hcol = scol[sct] * 128
                scol[sct] += 1
                nc.tensor.matmul(z_ps[:, ts(j, 32)],
                                 lhsT=sbuf[sct][0:65, 0, hcol:hcol + 128],
                                 rhs=WemA[:], start=False, stop=(j == 3))
            # gelu (tanh approximation, exact formula)
            zt = wp.tile([128, 4, 32], F32, tag="zt", name="zt")
            nc.scalar.activation(zt[:], z_ps[:].rearrange("p (c f) -> p c f",
                                                          f=32), AF.Copy)
            sqz = wp.tile([128, 4, 32], F32, tag="sqz", name="sqz")
            nc.scalar.activation(sqz[:], zt[:], AF.Square)
            nc.vector.tensor_scalar(sqz[:], sqz[:], 0.044715, 1.0, ALU.mult,
                                    ALU.add)
            u = wp.tile([128, 4, 32], F32, tag="ug", name="ug")
            nc.vector.tensor_tensor(out=u[:], in0=zt[:], in1=sqz[:],
                                    op=ALU.mult)
            tt = wp.tile([128, 4, 32], F32, tag="tt", name="tt")
            nc.scalar.activation(tt[:], u[:], AF.Tanh, scale=0.7978845608)
            nc.vector.tensor_scalar(tt[:], tt[:], 1.0, 0.5, ALU.add, ALU.mult)
            g_out = wp.tile([128, 4, 32], F32, tag="g_out", name="g_out")
            nc.vector.tensor_tensor(out=g_out[:], in0=zt[:], in1=tt[:],
                                    op=ALU.mult)
            x = wp.tile([128, 4, 32], F32, tag="x_e", name="x_e")
            nc.vector.tensor_tensor(
                out=x[:],
                in0=em_ps[:].rearrange("p (c f) -> p c f", f=32),
                in1=g_out[:], op=ALU.add)
            nc.vector.tensor_tensor(
                out=x[:], in0=x[:],
                in1=Be[:, None, :].to_broadcast([128, 4, 32]), op=ALU.add)
            xh = wp.tile([128, 4, 32], BF16, tag="xh", name="xh")
            self._layernorm(x[:], xh[:], 32, None, None)
            nc.sync.dma_start(
                out=e_wr[base // 4:base // 4 + 128, :],
                in_=xh[:].rearrange("p c f -> p (c f)"))

    def _output(self):
        nc, cfg = self.nc, self.cfg
        T0, rem = cfg.NLOC // 128, cfg.NLOC % 128
        if T0:
            nc.sync.dma_start(
                out=self.out_h[0:T0 * 128, :].rearrange("(t p) d -> p t d",
                                                        p=128),
                in_=self.h_sb[:, 0:T0, :])
        if rem:
            nc.sync.dma_start(out=self.out_h[T0 * 128:cfg.NLOC, :],
                              in_=self.h_sb[0:rem, T0, :])


# ------------------------------------------------------------------ runner ---
def make_in_maps(cfg, plan, inputs):
    W = {k: np.asarray(inputs[k]) for k in
         ("Wq", "Wk", "Wv", "Wo", "We", "Wem", "bem", "gn", "bn", "ge", "be")}
    fw = fold_weights(cfg, W)
    h0 = np.asarray(inputs["node_features"]).astype(np.float32)
    k0 = h0 @ np.asarray(W["Wk"][0], np.float32)
    v0 = h0 @ np.asarray(W["Wv"][0], np.float32)
    kv0 = _bf(np.concatenate([k0, v0], 1))
    q0 = h0 @ (np.asarray(W["Wq"][0], np.float32) * 0.25)
    zb0 = np.zeros((cfg.N, cfg.DE), np.float32)
    iota = _bf(np.tile(np.arange(128, dtype=np.float32)[None, :], (128, 1)))
    ones = np.ones((1, 128), np.float32)
    hbfi = np.zeros((cfg.NLOC, 128), np.float32)
    hbfi[:, 64] = 1.0
    in_maps = []
    for c in range(cfg.C):
        lo, hi = c * cfg.NLOC, (c + 1) * cfg.NLOC
        hl = np.zeros((cfg.NLOCP, 64), np.float32)
        hl[0:cfg.NLOC] = h0[lo:hi]
        dstq0 = np.zeros((cfg.NLOCP, 128), np.float32)
        dstq0[0:cfg.NLOC, 0:64] = q0[lo:hi]
        dstq0[0:cfg.NLOC, 64:96] = zb0[lo:hi]
        m = {
            "h_loc": hl, "kv0": kv0, "dstq0": _bf(dstq0), "e0": plan.e0[c],
            "hbfi": _bf(hbfi),
            "swl": plan.src_widx[c][0], "swh": plan.src_widx[c][1],
            "dw": plan.dst_widx[c],
            "dcol": _bf(plan.dstcol[c].reshape(-1, 128).T.copy()),
            "iota": iota, "onesrow": ones,
        }
        m.update(fw)
        in_maps.append(m)
    return in_maps


def run_model(cfg, inputs, trace=False):
    plan = Plan(cfg, inputs["edge_index"], inputs["edge_features"])
    nc = Builder(cfg, plan).build()
    in_maps = make_in_maps(cfg, plan, inputs)
    res = run_bass_kernel_spmd(nc, in_maps, list(range(cfg.C)), trace=trace)
    out = np.concatenate([res.results[c]["out"] for c in range(cfg.C)], 0)
    return out[0:cfg.N].astype(np.float32), res


def kernel(**inputs):
    out, _ = run_model(Cfg(), inputs)
    return out



# revision 3
# speedup vs baseline: 1.0314x; 1.0314x over previous
"""Trainium2 Bass kernel v2 for nn_DELTAModel: transposed-e pipeline,
gather-minimized (2xEP descriptors total), SPMD across 8 cores.

Passes: A0 (attn layer0, fully pre-staged streams, no gathers),
F1 = [e-update0 + attn1], F2 = [e-update1 + attn2]; each F pass does ONE
512B-row gather per edge fetching [k|v|ha]. dst-side q/hb come from PE
one-hot matmuls (OT) against SBUF-resident per-block tables. Segment sums
via one-hot matmuls (O). O/OT/eT0/kv0g are host-built DRAM streams.
e state is stored feature-major (eT tiles [128=4ch x 32f, 128 edges]);
gelu+LN run as flat ops with stats via PE SecBcast matmuls.
"""

import os
import sys

for _p in ("/root/pylib", "/opt/trn_rl_repo"):
    if os.path.isdir(_p) and _p not in sys.path:
        sys.path.append(_p)

import numpy as np
import ml_dtypes

import concourse.bacc as bacc
import concourse.mybir as mybir
import concourse.tile as tile
from concourse.bass import ts
from concourse.bass_utils import run_bass_kernel_spmd
from concourse.masks import make_identity

BF16 = mybir.dt.bfloat16
F32 = mybir.dt.float32
I16 = mybir.dt.int16
AF = mybir.ActivationFunctionType
ALU = mybir.AluOpType
AX = mybir.AxisListType


class Cfg:
    def __init__(self):
        self.N, self.E, self.D, self.DE, self.H, self.L, self.C = \
            50000, 1600000, 64, 32, 4, 3, 8
        self.SPLIT = 32768
        self.NLOC = self.N // self.C          # 6250
        self.NBLK = (self.NLOC + 127) // 128  # 49
        self.NLOCP = self.NBLK * 128


def _bf(x):
    return np.ascontiguousarray(np.asarray(x)).astype(ml_dtypes.bfloat16)


def _wrap_idx(a):
    n = a.shape[0]
    w = a.reshape(n // 16, 16).T.astype(np.int16)
    return np.tile(w, (8, 1))


class Plan:
    """Edge-cut by dst. Chunk = 128 edge slots, (block, section)-pure.
    nchunk[b,s] = max over cores of ceil(count/128); total rounded to %4
    (window integrity). Windows = 4 chunks; superwindows = 4 windows."""

    def __init__(self, cfg, edge_index):
        C, NLOC, SPLIT, NBLK = cfg.C, cfg.NLOC, cfg.SPLIT, cfg.NBLK
        src = np.asarray(edge_index[0]).astype(np.int64)
        dst = np.asarray(edge_index[1]).astype(np.int64)
        self.src_glob = src
        core = dst // NLOC
        self.cfg = cfg
        nchunk = np.zeros((NBLK, 2), np.int64)
        per_core = []
        for c in range(C):
            ids = np.nonzero(core == c)[0]
            dl = dst[ids] - c * NLOC
            sec = (src[ids] >= SPLIT).astype(np.int64)
            order = np.lexsort((dl, sec, dl // 128))
            ids, dl, sec = ids[order], dl[order], sec[order]
            per_core.append((ids, dl, sec, dl // 128))
            for b in range(NBLK):
                m = (dl // 128) == b
                for s in (0, 1):
                    cnt = int(np.count_nonzero(m & (sec == s)))
                    nchunk[b, s] = max(nchunk[b, s], (cnt + 127) // 128)
        pad = (-int(nchunk.sum())) % 4
        nchunk[NBLK - 1, 1] += pad
        self.nchunk = nchunk
        self.NCH = int(nchunk.sum())
        self.NWIN = self.NCH // 4
        self.EP = self.NCH * 128

        # chunk list + runs
        self.chunks = []          # (b, s, gcol)  gcol = col within block buf
        self.runs = []            # (b, s, gcol0, nch, ch0)
        self.blk_nch = [0] * NBLK
        for b in range(NBLK):
            gcol = 0
            for s in (0, 1):
                n = int(nchunk[b, s])
                if n == 0:
                    continue
                self.runs.append((b, s, gcol, n, len(self.chunks)))
                for _ in range(n):
                    self.chunks.append((b, s, gcol))
                    gcol += 1
            self.blk_nch[b] = gcol
        self.maxnch = max(self.blk_nch)
        assert min(self.blk_nch) >= 16, self.blk_nch
        # idx column offsets per run within swl/swh
        off = [0, 0]
        self.run_icol = []
        for (b, s, g0, n, c0) in self.runs:
            self.run_icol.append(off[s])
            off[s] += n * 8
        self.icol_tot = off
        # block chunk counts for U psum stop flags
        self.blk_chunks = [int(nchunk[b, 0] + nchunk[b, 1]) for b in range(NBLK)]

        # per-core slot assignment
        self.slot_edge = []   # edge id per slot (-1 pad)
        self.slot_dc = []     # dst col in block (-1 pad)
        self.srcw = []        # (swl, swh) wrapped idx arrays
        ch_off = np.zeros((NBLK, 2), np.int64)
        t = 0
        for b in range(NBLK):
            for s in (0, 1):
                ch_off[b, s] = t
                t += int(nchunk[b, s])
        for c in range(C):
            ids, dl, sec, blk = per_core[c]
            se = np.full(self.EP, -1, np.int64)
            dc = np.full(self.EP, -1, np.int64)
            si = [np.zeros(int(nchunk[:, 0].sum()) * 128, np.int16),
                  np.zeros(int(nchunk[:, 1].sum()) * 128, np.int16)]
            soff = [0, 0]
            for b in range(NBLK):
                for s in (0, 1):
                    m = (blk == b) & (sec == s)
                    k = int(np.count_nonzero(m))
                    o = int(ch_off[b, s]) * 128
                    se[o:o + k] = ids[m]
                    dc[o:o + k] = dl[m] - 128 * b
                    nsl = int(nchunk[b, s]) * 128
                    si[s][soff[s]:soff[s] + k] = \
                        (src[ids[m]] - SPLIT * s).astype(np.int16)
                    soff[s] += nsl
            self.slot_edge.append(se)
            self.slot_dc.append(dc)
            self.srcw.append((_wrap_idx(si[0]) if si[0].size else
                              np.zeros((128, 1), np.int16),
                              _wrap_idx(si[1]) if si[1].size else
                              np.zeros((128, 1), np.int16)))

    def build_streams(self, c, e0, kv0):
        """Host arrays for core c: kv0g, eT0, OOT."""
        cfg = self.cfg
        se, dc = self.slot_edge[c], self.slot_dc[c]
        real = se >= 0
        # kv0g [NCH*128, 128]
        kv0g = np.zeros((self.EP, 128), np.float32)
        kv0g[real] = kv0[self.src_glob[se[real]]]
        # eT0 [NWIN*128, 128]: row w*128 + 32*jj + f, col p
        ef = np.zeros((self.EP, cfg.DE), np.float32)
        ef[real] = e0[se[real]]
        eT0 = (ef.reshape(self.NWIN, 4, 128, cfg.DE)
               .transpose(0, 1, 3, 2)           # w, jj, f, p
               .reshape(self.NWIN * 128, 128))
        # OOT [NWIN*128, 1024]
        oot = np.zeros((self.NWIN, 128, 1024), np.float32)
        dcw = dc.reshape(self.NWIN, 4, 128)
        w_i, j_i, p_i = np.nonzero(dcw >= 0)
        c_i = dcw[w_i, j_i, p_i]
        oot[w_i, p_i, j_i * 128 + c_i] = 1.0
        oot[w_i, c_i, 512 + j_i * 128 + p_i] = 1.0
        return _bf(kv0g), _bf(eT0), _bf(oot.reshape(self.NWIN * 128, 1024))


def fold_weights(cfg, W):
    DE = cfg.DE
    out = {}
    for l in range(cfg.L):
        g_prev = np.ones(DE, np.float32) if l == 0 else np.asarray(W["ge"][l - 1], np.float32)
        b_prev = np.zeros(DE, np.float32) if l == 0 else np.asarray(W["be"][l - 1], np.float32)
        We = np.asarray(W["We"][l], np.float32)
        bd = np.zeros((128, 16), np.float32)
        for a in range(4):
            bd[32 * a:32 * a + 32, 4 * a:4 * a + 4] = g_prev[:, None] * We
        out[f"WeF_{l}"] = _bf(bd)
        out[f"Wo_{l}"] = _bf(np.asarray(W["Wo"][l]))
        out[f"gn_{l}"] = np.asarray(W["gn"][l], np.float32)[None, :]
        out[f"bn_{l}"] = np.asarray(W["bn"][l], np.float32)[None, :]
    # F pass fp=1,2 does e-update (l=fp-1) + attn (l=fp)
    for fp in (1, 2):
        lu = fp - 1
        g_in = np.ones(DE, np.float32) if lu == 0 else np.asarray(W["ge"][lu - 1], np.float32)
        b_in = np.zeros(DE, np.float32) if lu == 0 else np.asarray(W["be"][lu - 1], np.float32)
        Wem = np.asarray(W["Wem"][lu], np.float32)
        Cm = Wem[2 * cfg.D:]
        bd = np.zeros((128, 128), np.float32)
        for a in range(4):
            bd[32 * a:32 * a + 32, 32 * a:32 * a + 32] = g_in[:, None] * Cm
        out[f"WemCT_{fp}"] = _bf(bd)
        gb = np.asarray(W["bem"][lu], np.float32) + b_in @ Cm
        out[f"gelb_{fp}"] = np.tile(gb, 4)[:, None].astype(np.float32)   # [128,1]
        out[f"gvec_{fp}"] = np.tile(g_in, 4)[:, None].astype(np.float32)
        out[f"bvec_{fp}"] = np.tile(b_in, 4)[:, None].astype(np.float32)
        # table RHS for gen fp: [Wk|Wv|A_{fp-1}|Wq*0.25|B_{fp-1}]  [64,256]
        A = Wem[:cfg.D]
        B = Wem[cfg.D:2 * cfg.D]
        rhs = np.concatenate(
            [np.asarray(W["Wk"][fp], np.float32),
             np.asarray(W["Wv"][fp], np.float32), A,
             np.asarray(W["Wq"][fp], np.float32) * 0.25, B], axis=1)
        out[f"RHS_{fp}"] = _bf(rhs)
    sb = np.zeros((128, 128), np.float32)
    for a in range(4):
        sb[32 * a:32 * a + 32, 32 * a:32 * a + 32] = 1.0 / 32.0
    out["SecB"] = _bf(sb)
    out["identb"] = _bf(np.eye(128, dtype=np.float32))
    return out


class Builder:
    def __init__(self, cfg, plan):
        self.cfg, self.plan = cfg, plan
        self.nc = bacc.Bacc("TRN2")

    def declare(self):
        nc, cfg, plan = self.nc, self.cfg, self.plan
        dram = lambda n, s, d, k="ExternalInput": nc.dram_tensor(n, s, d, kind=k)
        self.in_h = dram("h_loc", [cfg.NLOCP, 64], F32)
        self.in_q0 = dram("q0", [cfg.NLOCP, 64], BF16)
        self.in_kv0g = dram("kv0g", [plan.EP, 128], BF16)
        self.in_eT0 = dram("eT0", [plan.NWIN * 128, 128], BF16)
        self.in_oot = dram("oot", [plan.NWIN * 128, 1024], BF16)
        self.in_swl = dram("swl", [128, max(plan.icol_tot[0], 8)], I16)
        self.in_swh = dram("swh", [128, max(plan.icol_tot[1], 8)], I16)
        self.in_ones = dram("onesrow", [1, 128], F32)
        self.w = {}
        wspec = [("SecB", [128, 128], BF16), ("identb", [128, 128], BF16)]
        for l in range(cfg.L):
            wspec += [(f"WeF_{l}", [128, 16], BF16), (f"Wo_{l}", [64, 64], BF16),
                      (f"gn_{l}", [1, 64], F32), (f"bn_{l}", [1, 64], F32)]
        for fp in (1, 2):
            wspec += [(f"WemCT_{fp}", [128, 128], BF16),
                      (f"gelb_{fp}", [128, 1], F32),
                      (f"gvec_{fp}", [128, 1], F32),
                      (f"bvec_{fp}", [128, 1], F32),
                      (f"RHS_{fp}", [64, 256], BF16)]
        for nm, sh, dt in wspec:
            self.w[nm] = dram(nm, sh, dt)
        self.out_h = dram("out", [cfg.NLOC, 64], F32, k="ExternalOutput")

    def build(self):
        nc, cfg = self.nc, self.cfg
        self.declare()
        with tile.TileContext(nc, num_cores=cfg.C) as tc:
            self.tc = tc
            with tc.tile_pool(name="persist", bufs=1) as pp, \
                 tc.tile_pool(name="dramp", bufs=1, space="DRAM") as dp, \
                 tc.tile_pool(name="work", bufs=3) as wp, \
                 tc.tile_pool(name="gath", bufs=1) as gp:
                self.pp, self.dp, self.wp, self.gp = pp, dp, wp, gp
                self._persistent()
                self._internal_dram()
                stop = os.environ.get("K_STOP", "")
                seq = ["a0", "n0", "f1", "n1", "f2", "n2"]
                cut = seq.index(stop) if stop in seq else len(seq) - 1
                go = lambda tag: seq.index(tag) <= cut
                if go("a0"):
                    self._a0()
                if go("n0"):
                    self._node_update(0, gen=1)
                if go("f1"):
                    self._fpass(1, self.in_eT0, self.kvha[0], self.e_dev)
                if go("n1"):
                    self._node_update(1, gen=2)
                if go("f2"):
                    self._fpass(2, self.e_dev, self.kvha[1], None)
                if go("n2"):
                    self._node_update(2, gen=None)
                self._output()
        nc.compile()
        return nc

    # ---- persistent ----
    def _persistent(self):
        nc, cfg, pp = self.nc, self.cfg, self.pp
        NBLK = cfg.NBLK
        self.h_sb = pp.tile([128, NBLK, 64], F32, name="h_sb")
        nc.sync.dma_start(out=self.h_sb[:],
                          in_=self.in_h[:].rearrange("(t p) d -> p t d", p=128))
        self.U_sb = pp.tile([128, NBLK, 68], F32, name="U_sb")
        self.QHB = pp.tile([128, NBLK, 96], BF16, name="QHB")
        nc.sync.dma_start(out=self.QHB[:, :, 0:64],
                          in_=self.in_q0[:].rearrange("(t p) d -> p t d", p=128))
        self.ident = pp.tile([128, 128], F32, name="ident")
        make_identity(nc, self.ident[:])
        self.ones_sb = pp.tile([1, 128], F32, name="ones_sb")
        nc.sync.dma_start(out=self.ones_sb[:], in_=self.in_ones[:])
        self.eps512 = pp.tile([1, 512], F32, name="eps512")
        nc.vector.memset(self.eps512[:], 1e-5)
        self.wsb = {}
        for nm, t in self.w.items():
            tt_ = pp.tile(list(t.shape), t.dtype, name=f"sb_{nm}")
            nc.sync.dma_start(out=tt_[:], in_=t[:])
            self.wsb[nm] = tt_
        # broadcast gn/bn to [128,64] via ones matmul
        qp_ctx = self.tc.tile_pool(name="ps_init", bufs=1, space="PSUM")
        qp = qp_ctx.__enter__()
        self.GB = {}
        for l in range(cfg.L):
            for nm in (f"gn_{l}", f"bn_{l}"):
                ps = qp.tile([128, 64], F32, space="PSUM", name=f"ps_{nm}",
                             tag="gbps")
                nc.tensor.matmul(ps[:], lhsT=self.ones_sb[:],
                                 rhs=self.wsb[nm][:], start=True, stop=True)
                sb = pp.tile([128, 64], F32, name=f"gb_{nm}")
                nc.scalar.activation(sb[:], ps[:], AF.Copy)
                self.GB[nm] = sb
        qp_ctx.__exit__(None, None, None)

    def _internal_dram(self):
        cfg, dp = self.cfg, self.dp
        self.kvha = [dp.tile([cfg.N, 256], BF16, name=f"kvha{g}",
                             addr_space="Shared") for g in (1, 2)]
        self.kvha_loc = dp.tile([cfg.NLOC, 256], BF16, name="kvha_loc")
        self.e_dev = dp.tile([self.plan.NWIN * 128, 128], BF16, name="e_dev")

    # ---- shared per-window attention tail ----
    def _attn_tail(self, wch, wg, qdps, kbuf, kcol, WeF, lhsT_bias, l):
        """wch: 4 (b,s,gcol); kbuf(j): tile for chunk j; kcol(j): col.
        lhsT_bias: [128,128] AP for bias matmul. qdps [128,4,68] psum."""
        nc, wp, plan = self.nc, self.wp, self.plan
        # qd per chunk + bias; qdps [128, 5, 64]: rows 0-3 qd, row 4 bias
        for j, (b, s, g) in enumerate(wch):
            nc.tensor.matmul(qdps[:, j, :], lhsT=wg[:, 512 + j * 128:512 + (j + 1) * 128],
                             rhs=self.QHB[:, b, 0:64], start=True, stop=True)
        nc.tensor.matmul(qdps[:, 4, 0:16], lhsT=lhsT_bias, rhs=WeF[:],
                         start=True, stop=True)
        qk = wp.tile([128, 4, 64], BF16, tag="qk", name="qk", bufs=2)
        for (jj, kk) in self._runs4(wch):
            nc.vector.tensor_tensor(
                out=qk[:, jj:jj + kk, :], in0=qdps[:, jj:jj + kk, :],
                in1=kbuf(jj)[:, kcol(jj):kcol(jj) + kk, 0:64], op=ALU.mult)
        s_sb = wp.tile([128, 4, 4], F32, tag="s_sb", name="s_sb", bufs=2)
        nc.vector.tensor_reduce(
            out=s_sb[:], in_=qk[:].rearrange("p c (h x) -> p c h x", h=4),
            axis=AX.X, op=ALU.add)
        nc.vector.tensor_tensor(
            out=s_sb[:], in0=s_sb[:],
            in1=qdps[:, 4, 0:16].rearrange("p (c f) -> p c f", f=4),
            op=ALU.add)
        p_sb = wp.tile([128, 4, 4], BF16, tag="p_sb", name="p_sb", bufs=2)
        nc.scalar.activation(p_sb[:], s_sb[:], AF.Exp)
        M = wp.tile([128, 4, 68], BF16, tag="M", name="M", bufs=2)
        nc.scalar.activation(M[:, :, 64:68], p_sb[:], AF.Copy)
        for j in range(4):
            nc.vector.tensor_tensor(
                out=M[:, j, 0:64].rearrange("p (h x) -> p h x", h=4),
                in0=p_sb[:, j, :, None].to_broadcast([128, 4, 16]),
                in1=kbuf(j)[:, kcol(j):kcol(j) + 1, 64:128]
                    .rearrange("p c (h x) -> p (c h) x", h=4),
                op=ALU.mult)
        for j, (b, s, g) in enumerate(wch):
            seen = self._blk_seen.get(b, 0)
            if seen == 0:
                self._ups = self.qp.tile([128, 68], F32, space="PSUM",
                                         tag="U", bufs=2, name="ups")
            last = seen + 1 == plan.blk_chunks[b]
            nc.tensor.matmul(self._ups[:], lhsT=wg[:, j * 128:(j + 1) * 128],
                             rhs=M[:, j, :], start=(seen == 0), stop=last)
            self._blk_seen[b] = seen + 1
            if last:
                nc.scalar.activation(self.U_sb[:, b, :], self._ups[:], AF.Copy)

    @staticmethod
    def _runs4(wch):
        """group window chunks into runs of same (b,s) w/ consecutive gcol"""
        out = []
        j0 = 0
        for j in range(1, 5):
            if j == 4 or wch[j][0] != wch[j0][0] or wch[j][1] != wch[j0][1] \
               or wch[j][2] != wch[j0][2] + (j - j0):
                out.append((j0, j - j0))
                j0 = j
        return out

    # ---- A0 ----
    def _a0(self):
        nc, cfg, plan, wp, gp = self.nc, self.cfg, self.plan, self.wp, self.gp
        qp_ctx = self.tc.tile_pool(name="ps_a0", bufs=1, space="PSUM")
        self.qp = qp_ctx.__enter__()
        WeF = self.wsb["WeF_0"]
        self._blk_seen = {}
        for w0 in range(0, plan.NWIN, 4):
            nw = min(4, plan.NWIN - w0)
            nch = nw * 4
            kvs = gp.tile([128, 16, 128], BF16, tag="kvs", bufs=2, name="kvs")
            nc.sync.dma_start(
                out=kvs[:, 0:nch, :],
                in_=self.in_kv0g[w0 * 512:w0 * 512 + nch * 128, :]
                    .rearrange("(t p) d -> p t d", p=128))
            eTs = gp.tile([128, 4, 128], BF16, tag="eTs", bufs=2, name="eTs")
            nc.sync.dma_start(
                out=eTs[:, 0:nw, :],
                in_=self.in_eT0[w0 * 128:(w0 + nw) * 128, :]
                    .rearrange("(t p) d -> p t d", p=128))
            oot = gp.tile([128, 4, 1024], BF16, tag="oot", bufs=2, name="oot")
            nc.sync.dma_start(
                out=oot[:, 0:nw, :],
                in_=self.in_oot[w0 * 128:(w0 + nw) * 128, :]
                    .rearrange("(t p) d -> p t d", p=128))
            for wl in range(nw):
                w = w0 + wl
                wch = [plan.chunks[w * 4 + j] for j in range(4)]
                qdps = self.qp.tile([128, 5, 64], F32, space="PSUM",
                                    tag="qdps", bufs=2, name="qdps")
                self._attn_tail(
                    wch, oot[:, wl, :], qdps,
                    kbuf=lambda j, _w=wl: kvs, kcol=lambda j, _w=wl: _w * 4 + j,
                    WeF=WeF, lhsT_bias=eTs[:, wl, :], l=0)
        qp_ctx.__exit__(None, None, None)

    # ---- fused pass ----
    def _fpass(self, fp, eT_src, kvha_tab, e_wr):
        nc, cfg, plan, wp, gp = self.nc, self.cfg, self.plan, self.wp, self.gp
        qp_ctx = self.tc.tile_pool(name=f"ps_f{fp}", bufs=1, space="PSUM")
        self.qp = qp_ctx.__enter__()
        qp = self.qp
        WeF = self.wsb[f"WeF_{fp}"]
        WemCT = self.wsb[f"WemCT_{fp}"]
        gelb = self.wsb[f"gelb_{fp}"]
        identb = self.wsb["identb"]
        SecB = self.wsb["SecB"]
        self._blk_seen = {}
        gbufs = {}

        def ensure_gather(b):
            if b in gbufs:
                return
            gb = gp.tile([128, plan.maxnch, 256], BF16, tag="gb", bufs=2,
                         name="gb")
            for (rb, s, g0, n, c0), icol in zip(plan.runs, plan.run_icol):
                if rb != b:
                    continue
                it = gp.tile([128, plan.maxnch * 8], I16, tag="gidx", bufs=2,
                             name="gidx")
                widx = self.in_swl if s == 0 else self.in_swh
                nc.sync.dma_start(out=it[:, 0:n * 8],
                                  in_=widx[:, icol:icol + n * 8])
                tab = (kvha_tab[0:cfg.SPLIT, :] if s == 0 else
                       kvha_tab[cfg.SPLIT:cfg.N, :])
                nc.gpsimd.dma_gather(gb[:, g0:g0 + n, :], tab, it[:, 0:n * 8],
                                     n * 128, n * 128, 256, single_packet=False)
            gbufs[b] = gb

        for w0 in range(0, plan.NWIN, 4):
            nw = min(4, plan.NWIN - w0)
            sw_ch = [plan.chunks[w * 4 + j] for w in range(w0, w0 + nw)
                     for j in range(4)]
            for (b, s, g) in sw_ch:
                ensure_gather(b)
            eTs = gp.tile([128, 4, 128], BF16, tag="eTs", bufs=2, name="eTs")
            nc.sync.dma_start(
                out=eTs[:, 0:nw, :],
                in_=eT_src[w0 * 128:(w0 + nw) * 128, :]
                    .rearrange("(t p) d -> p t d", p=128))
            oot = gp.tile([128, 4, 1024], BF16, tag="oot", bufs=2, name="oot")
            nc.sync.dma_start(
                out=oot[:, 0:nw, :],
                in_=self.in_oot[w0 * 128:(w0 + nw) * 128, :]
                    .rearrange("(t p) d -> p t d", p=128))
            NE = nw * 128
            # -- e-part --
            if fp == 1:
                eTgb = eTs  # raw e0 (g=1,b=0)
            else:
                eTgb = wp.tile([128, 4, 128], BF16, tag="eTgb", name="eTgb", bufs=2)
                nc.vector.tensor_scalar(
                    eTgb[:, 0:nw, :], eTs[:, 0:nw, :],
                    self.wsb[f"gvec_{fp}"][:], self.wsb[f"bvec_{fp}"][:],
                    ALU.mult, ALU.add)
            zT = qp.tile([128, 4, 128], F32, space="PSUM", tag="zT", bufs=1,
                         name="zT")
            nc.tensor.matmul(
                zT[:, 0:nw, :].rearrange("p c f -> p (c f)"),
                lhsT=WemCT[:],
                rhs=eTgb[:, 0:nw, :].rearrange("p c f -> p (c f)"),
                start=True, stop=False)
            for wl in range(nw):
                wch = [plan.chunks[(w0 + wl) * 4 + j] for j in range(4)]
                # haT: copy gather cols 128:160 to contiguous tile, transpose
                ha_sb = wp.tile([128, 4, 32], BF16, tag="ha_sb", name="ha_sb", bufs=2)
                for (jj, kk) in self._runs4(wch):
                    b, s, g = wch[jj]
                    nc.scalar.activation(ha_sb[:, jj:jj + kk, :],
                                         gbufs[b][:, g:g + kk, 128:160],
                                         AF.Copy)
                nc.tensor.matmul(
                    zT[:, wl, :], lhsT=ha_sb[:].rearrange("p c f -> p (c f)"),
                    rhs=identb[:], start=False, stop=False)
                # hb via OT then transpose back
                hbps = qp.tile([128, 4, 32], F32, space="PSUM", tag="hbps",
                               bufs=1, name="hbps")
                for j, (b, s, g) in enumerate(wch):
                    nc.tensor.matmul(
                        hbps[:, j, :],
                        lhsT=oot[:, wl, 512 + j * 128:512 + (j + 1) * 128],
                        rhs=self.QHB[:, b, 64:96], start=True, stop=True)
                hb_sb = wp.tile([128, 4, 32], BF16, tag="hb_sb", name="hb_sb", bufs=2)
                nc.scalar.activation(hb_sb[:], hbps[:], AF.Copy)
                nc.tensor.matmul(
                    zT[:, wl, :],
                    lhsT=hb_sb[:].rearrange("p c f -> p (c f)"),
                    rhs=identb[:], start=False, stop=(wl == nw - 1))
            zf = zT[:, 0:nw, :].rearrange("p c f -> p (c f)")
            # gelu: x = e + 0.5z + 0.5z*tanh(0.79788*(z + 0.044715 z^3))
            zsb = wp.tile([128, 4 * 128], BF16, tag="zsb", name="zsb", bufs=2)
            nc.scalar.activation(zsb[:, 0:NE], zf, AF.Identity, bias=gelb[:])
            sq = wp.tile([128, 4 * 128], BF16, tag="sqz", name="sqz", bufs=2)
            nc.scalar.activation(sq[:, 0:NE], zsb[:, 0:NE], AF.Square,
                                 scale=0.2114626)
            p3 = wp.tile([128, 4 * 128], BF16, tag="tg", name="p3", bufs=2)
            nc.vector.tensor_tensor(out=p3[:, 0:NE], in0=zsb[:, 0:NE],
                                    in1=sq[:, 0:NE], op=ALU.mult)
            ug = wp.tile([128, 4 * 128], BF16, tag="ug", name="ug", bufs=2)
            nc.vector.tensor_tensor(out=ug[:, 0:NE], in0=zsb[:, 0:NE],
                                    in1=p3[:, 0:NE], op=ALU.add)
            th = wp.tile([128, 4 * 128], BF16, tag="th", name="th", bufs=2)
            nc.scalar.activation(th[:, 0:NE], ug[:, 0:NE], AF.Tanh,
                                 scale=0.7978845608)
            zh = wp.tile([128, 4 * 128], BF16, tag="zh", name="zh", bufs=2)
            nc.scalar.activation(zh[:, 0:NE], zsb[:, 0:NE], AF.Copy, scale=0.5)
            gl = wp.tile([128, 4 * 128], BF16, tag="gl", name="gl", bufs=2)
            nc.vector.tensor_tensor(out=gl[:, 0:NE], in0=zh[:, 0:NE],
                                    in1=th[:, 0:NE], op=ALU.mult)
            x1 = wp.tile([128, 4 * 128], BF16, tag="x1", name="x1", bufs=2)
            nc.vector.tensor_tensor(
                out=x1[:, 0:NE], in0=zh[:, 0:NE],
                in1=eTgb[:, 0:nw, :].rearrange("p c f -> p (c f)"), op=ALU.add)
            x = wp.tile([128, 4 * 128], BF16, tag="x_e", name="x_e", bufs=2)
            nc.vector.tensor_tensor(out=x[:, 0:NE], in0=x1[:, 0:NE],
                                    in1=gl[:, 0:NE], op=ALU.add)
            sqx = wp.tile([128, 4 * 128], BF16, tag="sqx", name="sqx", bufs=2)
            nc.scalar.activation(sqx[:, 0:NE], x[:, 0:NE], AF.Square)
            S1 = qp.tile([128, 4 * 128], F32, space="PSUM", tag="S1", bufs=1,
                         name="S1")
            nc.tensor.matmul(S1[:, 0:NE], lhsT=SecB[:], rhs=x[:, 0:NE],
                             start=True, stop=True)
            S2 = qp.tile([128, 4 * 128], F32, space="PSUM", tag="S2", bufs=1,
                         name="S2")
            nc.tensor.matmul(S2[:, 0:NE], lhsT=SecB[:], rhs=sqx[:, 0:NE],
                             start=True, stop=False)
            nc.tensor.matmul(S2[:, 0:NE], lhsT=self.ones_sb[:],
                             rhs=self.eps512[:, 0:NE], start=False, stop=True)
            mu = wp.tile([128, 4 * 128], BF16, tag="mu", name="mu", bufs=2)
            nc.scalar.activation(mu[:, 0:NE], S1[:, 0:NE], AF.Copy)
            mu2 = wp.tile([128, 4 * 128], F32, tag="mu2", name="mu2", bufs=2)
            nc.vector.tensor_tensor(out=mu2[:, 0:NE], in0=mu[:, 0:NE],
                                    in1=mu[:, 0:NE], op=ALU.mult)
            vv = wp.tile([128, 4 * 128], F32, tag="vv", name="vv", bufs=2)
            nc.vector.tensor_tensor(out=vv[:, 0:NE], in0=S2[:, 0:NE],
                                    in1=mu2[:, 0:NE], op=ALU.subtract)
            vr = wp.tile([128, 4 * 128], F32, tag="vr", name="vr", bufs=2)
            nc.vector.reciprocal_approx_fast(out=vr[:, 0:NE], in_=vv[:, 0:NE])
            rstd = wp.tile([128, 4 * 128], BF16, tag="rstd", name="rstd", bufs=2)
            nc.scalar.activation(rstd[:, 0:NE], vr[:, 0:NE], AF.Sqrt)
            xn = wp.tile([128, 4, 128], BF16, tag="xn", name="xn", bufs=2)
            xnf = xn[:].rearrange("p c f -> p (c f)")
            nc.vector.tensor_tensor(out=xnf[:, 0:NE], in0=x[:, 0:NE],
                                    in1=mu[:, 0:NE], op=ALU.subtract)
            nc.vector.tensor_tensor(out=xnf[:, 0:NE], in0=xnf[:, 0:NE],
                                    in1=rstd[:, 0:NE], op=ALU.mult)
            if e_wr is not None:
                nc.sync.dma_start(
                    out=e_wr[w0 * 128:(w0 + nw) * 128, :]
                        .rearrange("(t p) d -> p t d", p=128),
                    in_=xn[:, 0:nw, :])
            # -- a-part --
            for wl in range(nw):
                wch = [plan.chunks[(w0 + wl) * 4 + j] for j in range(4)]
                qdps = qp.tile([128, 5, 64], F32, space="PSUM", tag="qdps",
                               bufs=2, name="qdps")
                self._attn_tail(
                    wch, oot[:, wl, :], qdps,
                    kbuf=lambda j, _w=wch: gbufs[_w[j][0]],
                    kcol=lambda j, _w=wch: _w[j][2],
                    WeF=WeF, lhsT_bias=xn[:, wl, :], l=fp)
            # free gbufs of completed blocks
            done = set(b for b in gbufs if self._blk_seen.get(b, 0) ==
                       plan.blk_chunks[b])
            for b in done:
                del gbufs[b]
        qp_ctx.__exit__(None, None, None)

    # ---- node update + tables ----
    def _node_update(self, l, gen):
        nc, cfg = self.nc, self.cfg
        qp_ctx = self.tc.tile_pool(name=f"ps_n{l}", bufs=2, space="PSUM")
        qp = qp_ctx.__enter__()
        wp = self.wp
        NBLK = cfg.NBLK
        U, h = self.U_sb, self.h_sb
        rec = wp.tile([128, NBLK, 4], F32, tag="rec", name="rec", bufs=1)
        nc.vector.tensor_scalar(rec[:], U[:, :, 64:68], 1e-9, None, ALU.add)
        nc.vector.reciprocal(rec[:], rec[:])
        agg = wp.tile([128, NBLK, 64], F32, tag="agg", name="agg", bufs=1)
        nc.vector.tensor_tensor(
            out=agg[:].rearrange("p t (h x) -> p t h x", h=4),
            in0=U[:, :, 0:64].rearrange("p t (h x) -> p t h x", h=4),
            in1=rec[:, :, :, None].to_broadcast([128, NBLK, 4, 16]),
            op=ALU.mult)
        x = agg  # reuse buffer: agg[:, t] dead after its transpose
        Wo = self.wsb[f"Wo_{l}"]
        for t in range(NBLK):
            tp = qp.tile([128, 128], F32, space="PSUM", tag="tp", name="tp")
            nc.tensor.transpose(tp[0:64, :], agg[:, t, :], self.ident[:])
            aggT = wp.tile([64, 128], BF16, tag="aggT", name="aggT")
            nc.scalar.activation(aggT[:], tp[0:64, :], AF.Copy)
            rp = qp.tile([128, 64], F32, space="PSUM", tag="rp", name="rp")
            nc.tensor.matmul(rp[:], lhsT=aggT[:], rhs=Wo[:], start=True,
                             stop=True)
            nc.vector.tensor_tensor(out=x[:, t, :], in0=rp[:], in1=h[:, t, :],
                                    op=ALU.add)
        self._layernorm(x[:], h[:], 64, self.GB[f"gn_{l}"], self.GB[f"bn_{l}"])
        if gen is not None:
            RHS = self.wsb[f"RHS_{gen}"]
            kv_wr = self.kvha_loc
            for t in range(NBLK):
                rows = min(128, cfg.NLOC - t * 128)
                tp = qp.tile([128, 128], F32, space="PSUM", tag="tp",
                             name="tp")
                nc.tensor.transpose(tp[0:64, :], h[:, t, :], self.ident[:])
                hT = wp.tile([64, 128], BF16, tag="hT", name="hT")
                nc.scalar.activation(hT[:], tp[0:64, :], AF.Copy)
                kq = qp.tile([128, 256], F32, space="PSUM", tag="kq",
                             name="kq")
                nc.tensor.matmul(kq[0:rows, :], lhsT=hT[:, 0:rows], rhs=RHS[:],
                                 start=True, stop=True)
                kvt = wp.tile([128, 160], BF16, tag="kvt", name="kvt")
                nc.scalar.activation(kvt[0:rows, :], kq[0:rows, 0:160],
                                     AF.Copy)
                nc.sync.dma_start(
                    out=kv_wr[t * 128:t * 128 + rows, 0:160],
                    in_=kvt[0:rows, :])
                nc.scalar.activation(self.QHB[:, t, :], kq[:, 160:256],
                                     AF.Copy)
            nc.gpsimd.collective_compute(
                "AllGather", ALU.bypass, ins=[self.kvha_loc[:]],
                outs=[self.kvha[gen - 1][:]],
                replica_groups=[list(range(cfg.C))])
            self.tc.strict_bb_all_engine_barrier()
        qp_ctx.__exit__(None, None, None)

    def _layernorm(self, x, out, F, Gt, Bt):
        nc, wp = self.nc, self.wp
        T = x.shape[1]
        m1 = wp.tile([128, T], F32, tag="m1", name="m1", bufs=1)
        m2 = wp.tile([128, T], F32, tag="m2", name="m2", bufs=1)
        sq = wp.tile([128, T, F], F32, tag="sq", name="sq", bufs=1)
        nc.vector.tensor_reduce(out=m1[:, :, None], in_=x, axis=AX.X,
                                op=ALU.add)
        nc.scalar.activation(sq[:], x, AF.Square)
        nc.vector.tensor_reduce(out=m2[:, :, None], in_=sq[:], axis=AX.X,
                                op=ALU.add)
        mean = wp.tile([128, T], F32, tag="mean", name="mean", bufs=1)
        nc.vector.tensor_scalar(mean[:], m1[:], 1.0 / F, None, ALU.mult)
        v = wp.tile([128, T], F32, tag="vvar", name="vvar", bufs=1)
        nc.vector.tensor_tensor(out=v[:], in0=mean[:], in1=m1[:], op=ALU.mult)
        nc.vector.tensor_tensor(out=v[:], in0=m2[:], in1=v[:], op=ALU.subtract)
        nc.vector.tensor_scalar(v[:], v[:], 1.0 / F, 1e-5, ALU.mult, ALU.add)
        nc.vector.reciprocal(v[:], v[:])
        rstd = wp.tile([128, T], F32, tag="rstd_n", name="rstd_n", bufs=1)
        nc.scalar.activation(rstd[:], v[:], AF.Sqrt)
        xc = sq
        nc.vector.tensor_tensor(out=xc[:], in0=x,
                                in1=mean[:, :, None].to_broadcast([128, T, F]),
                                op=ALU.subtract)
        nc.vector.tensor_tensor(out=xc[:], in0=xc[:],
                                in1=rstd[:, :, None].to_broadcast([128, T, F]),
                                op=ALU.mult)
        nc.vector.tensor_tensor(
            out=xc[:], in0=xc[:],
            in1=Gt[:, None, :].to_broadcast([128, T, F]), op=ALU.mult)
        nc.vector.tensor_tensor(
            out=out, in0=xc[:],
            in1=Bt[:, None, :].to_broadcast([128, T, F]), op=ALU.add)

    def _output(self):
        nc, cfg = self.nc, self.cfg
        T0, rem = cfg.NLOC // 128, cfg.NLOC % 128
        if T0:
            nc.sync.dma_start(
                out=self.out_h[0:T0 * 128, :].rearrange("(t p) d -> p t d",
                                                        p=128),
                in_=self.h_sb[:, 0:T0, :])
        if rem:
            nc.sync.dma_start(out=self.out_h[T0 * 128:cfg.NLOC, :],
                              in_=self.h_sb[0:rem, T0, :])


# ---------------------------------------------------------------- runner ----
def make_in_maps(cfg, plan, inputs):
    W = {k: np.asarray(inputs[k]) for k in
         ("Wq", "Wk", "Wv", "Wo", "We", "Wem", "bem", "gn", "bn", "ge", "be")}
    fw = fold_weights(cfg, W)
    h0 = np.asarray(inputs["node_features"]).astype(np.float32)
    e0 = np.asarray(inputs["edge_features"]).astype(np.float32)
    k0 = h0 @ np.asarray(W["Wk"][0], np.float32)
    v0 = h0 @ np.asarray(W["Wv"][0], np.float32)
    kv0 = np.concatenate([k0, v0], 1)
    q0 = h0 @ (np.asarray(W["Wq"][0], np.float32) * 0.25)
    ones = np.ones((1, 128), np.float32)
    in_maps = []
    for c in range(cfg.C):
        lo, hi = c * cfg.NLOC, (c + 1) * cfg.NLOC
        hl = np.zeros((cfg.NLOCP, 64), np.float32)
        hl[0:cfg.NLOC] = h0[lo:hi]
        q0l = np.zeros((cfg.NLOCP, 64), np.float32)
        q0l[0:cfg.NLOC] = q0[lo:hi]
        kv0g, eT0, oot = plan.build_streams(c, e0, kv0)
        m = {"h_loc": hl, "q0": _bf(q0l), "kv0g": kv0g, "eT0": eT0,
             "oot": oot, "onesrow": ones,
             "swl": plan.srcw[c][0], "swh": plan.srcw[c][1]}
        m.update(fw)
        in_maps.append(m)
    return in_maps


def run_model(cfg, inputs, trace=False):
    plan = Plan(cfg, inputs["edge_index"])
    nc = Builder(cfg, plan).build()
    in_maps = make_in_maps(cfg, plan, inputs)
    res = run_bass_kernel_spmd(nc, in_maps, list(range(cfg.C)), trace=trace)
    out = np.concatenate([res.results[c]["out"] for c in range(cfg.C)], 0)
    return out[0:cfg.N].astype(np.float32), res


def kernel(**inputs):
    out, _ = run_model(Cfg(), inputs)
    return out
